# revision 30
# baseline (speedup 1.0000x reference)
"""Single-head attention (B=4, S=4096, E=2048, d=128) on 8 trn2 cores.

Sharding: core c handles (batch b = c//2, seq half h = c%2). Each core
projects q/k/v only for its own 2048-row half; the pair (2b, 2b+1)
exchanges K/V halves with a 2-core AllGather, overlapped with attention
over the own half. Host ships x already transposed (xT [E, HALF] bf16,
own half only). Attention runs in two passes: pass A over the 16 own
k-chunks (no dependency on the exchange, which overlaps it), pass B
over the 16 peer k-chunks (read from the
gather output via a runtime peer-slot register, host-supplied); partial
PV/denominator accumulations are combined in SBUF. Softmax over keys is
permutation-invariant, so per-core key order (own-first) is harmless.

Per-core pipeline (matmuls bf16, fp32 PSUM accumulation):
  xT tiles  <- plain DMA loads [128e x 1024s] x 32, both HWDGE queues
  k/v own   <- W-stationary projection into kv_own [k|v], bias folded
               into the ACT PSUM->SBUF evacuation (Identity activation)
  kv exchange: kv_own -> DRAM -> AllGather(pair) -> kv_peer
  per k-pair: scoresT[k, q] = kT_chunk^T @ qT  (2 matmuls into one
               [128 x 1024] PSUM tile), one Exp over both chunks
               (scale=1/sqrt(d) folded in; no max subtraction needed:
               scores are O(sigma~1), exp is fp32/bf16-safe), 2 PV
               matmuls accumulating out_T[d, q].
  softmax denominators: DVE pair-sum tree of exp tiles (depth 4),
               then exact ones-column matmul accumulation in PSUM.
Host: out = (out_T / sums).T per core, reassembled into [4,4096,128].
"""

import numpy as np
import ml_dtypes

import concourse.tile as tile
from concourse import bacc, mybir
from concourse.bass_utils import run_bass_kernel_spmd
from concourse.masks import make_identity

N_CORES = 8
B, S, E, D = 4, 4096, 2048, 128
HALF = S // 2  # queries / own keys per core
QB = 512  # query block (PSUM bank width in fp32)
SCALE = 1.0 / float(np.sqrt(D))

BF16 = mybir.dt.bfloat16
F32 = mybir.dt.float32
AF = mybir.ActivationFunctionType

_CACHE = {}


def _build():
    nc = bacc.Bacc(
        trn_type="TRN2", target_bir_lowering=False, debug=False, num_devices=N_CORES
    )

    x_d = nc.dram_tensor("xt", [E, HALF], BF16, kind="ExternalInput").ap()
    w_d = nc.dram_tensor(
        "w", [128, (E // 128) * 3 * D], BF16, kind="ExternalInput"
    ).ap()
    bias_d = nc.dram_tensor("bias_cols", [D, 3], F32, kind="ExternalInput").ap()
    peer_d = nc.dram_tensor("peer", [1, 1], mybir.dt.uint32, kind="ExternalInput").ap()
    out_d = nc.dram_tensor("out_t", [D, HALF], F32, kind="ExternalOutput").ap()
    sums_d = nc.dram_tensor("sums", [1, HALF], F32, kind="ExternalOutput").ap()

    NE = E // 128  # 16 e-chunks
    NQ = HALF // 1024  # 2 own s-quarters
    SQ = 1024
    NQB = HALF // QB  # 4 query blocks
    GROUPS = [[2 * i, 2 * i + 1] for i in range(N_CORES // 2)]

    with tile.TileContext(nc) as tc:
        with (
            tc.tile_pool(name="xt", bufs=32) as xt_pool,
            tc.tile_pool(name="wsb", bufs=1) as w_pool,
            tc.tile_pool(name="persist", bufs=1) as persist,
            tc.tile_pool(name="exp", bufs=20) as exp_pool,
            tc.tile_pool(name="comb", bufs=6) as comb_pool,
            tc.tile_pool(name="osb", bufs=3) as out_pool,
            tc.tile_pool(name="dram", bufs=1, space="DRAM") as dram_pool,
            tc.tile_pool(name="ps_big", bufs=3, space="PSUM") as ps_big,
            tc.tile_pool(name="ps_acc", bufs=1, space="PSUM") as ps_acc,
            tc.tile_pool(name="ps_small", bufs=1, space="PSUM") as ps_small,
        ):
            # ---- constants / small inputs (scalar HWDGE queue, parallel to x) ----
            w_sb = w_pool.tile([128, NE * 3 * D], BF16, tag="w")
            wg = NE * 3 * D // 4
            for g in range(4):
                nc.scalar.dma_start(
                    w_sb[:, g * wg : (g + 1) * wg], w_d[:, g * wg : (g + 1) * wg]
                )
            bias_sb = persist.tile([D, 3], F32, tag="bias")
            nc.scalar.dma_start(bias_sb[:], bias_d[:])
            ones_col = persist.tile([128, 1], BF16, tag="ones")
            nc.gpsimd.memset(ones_col[:], 1.0)
            ident = persist.tile([128, 128], BF16, tag="ident")
            make_identity(nc, ident[:])

            # peer slot register (host supplies 1 on even cores, 0 on odd)
            peer_reg = nc.sync.alloc_register("peer_slot")
            nc.sync.reg_load(peer_reg, peer_d[0:1, 0:1])
            peer_val = nc.sync.snap(peer_reg, donate=True, min_val=0, max_val=1)

            # ---- x loads, per (quarter, e-chunk), in consumption order ----
            xt = {}
            for sq in range(NQ):
                for e in range(NE):
                    t = xt_pool.tile([128, SQ], BF16, tag="xt")
                    eng = nc.sync if e % 2 == 0 else nc.scalar
                    eng.dma_start(
                        t[:], x_d[e * 128 : (e + 1) * 128, sq * SQ : (sq + 1) * SQ]
                    )
                    xt[(sq, e)] = t

            qT = persist.tile([D, HALF], BF16, tag="qT")
            kv_own = persist.tile([D, S], BF16, tag="kv_own")  # [k own | v own]
            kv_peer = persist.tile([D, S], BF16, tag="kv_peer")  # [k peer | v peer]
            v_sb = persist.tile([128, S // 128 * D], BF16, tag="v")
            sums_sb = persist.tile([1, HALF], F32, tag="sums_sb")
            o_stage = persist.tile([D, HALF], F32, tag="o_stage")

            def k_ap(k):  # kT chunk k (d on partitions)
                src, kk = (kv_own, k) if k < 16 else (kv_peer, k - 16)
                return src[:, kk * 128 : (kk + 1) * 128]

            def vt_ap(k):  # vT chunk k
                src, kk = (kv_own, k) if k < 16 else (kv_peer, k - 16)
                return src[:, HALF + kk * 128 : HALF + (kk + 1) * 128]

            def project(col_group, dst, dst_off, sq, bias_idx):
                """One quarter-wide (1024) projection block."""
                ps = ps_big.tile([128, SQ], F32, tag="ps_big")
                for e in range(NE):
                    w_ap = w_sb[
                        :, e * 3 * D + col_group * D : e * 3 * D + (col_group + 1) * D
                    ]
                    for half in range(2):
                        nc.tensor.matmul(
                            ps[:, half * QB : (half + 1) * QB],
                            lhsT=w_ap,
                            rhs=xt[(sq, e)][:, half * QB : (half + 1) * QB],
                            start=(e == 0),
                            stop=(e == NE - 1),
                        )
                nc.scalar.activation(
                    dst[:, dst_off : dst_off + SQ],
                    ps[:],
                    AF.Identity,
                    bias=bias_sb[:, bias_idx : bias_idx + 1],
                )

            def v_transpose(k):
                ps_t = ps_big.tile([128, 128], BF16, tag="ps_big")
                nc.tensor.transpose(ps_t[:], vt_ap(k), ident[:])
                nc.vector.tensor_copy(v_sb[:, k * D : (k + 1) * D], ps_t[:])

            def scores_exp(qb, kp):
                """Scores + exp for k-pair kp, query block qb -> exp tile."""
                q_ap = qT[:, qb * QB : (qb + 1) * QB]
                ps_s = ps_big.tile([128, 2 * QB], F32, tag="ps_big")
                for half in range(2):
                    nc.tensor.matmul(
                        ps_s[:, half * QB : (half + 1) * QB],
                        lhsT=k_ap(2 * kp + half),
                        rhs=q_ap,
                        start=True,
                        stop=True,
                    )
                ex = exp_pool.tile([128, 2 * QB], BF16, tag="exp")
                nc.scalar.activation(ex[:], ps_s[:], AF.Exp, scale=SCALE)
                return ex

            def pv_sums(qb, kp0, nkp, first, last, pre=None):
                """PV + denominator accumulation over nkp k-pairs from kp0.

                pre: already-emitted exp tiles (from scores_exp), else
                scores+exp are emitted inline per k-pair.
                """
                ps_o = ps_acc.tile([128, QB], F32, tag="ps_acc")
                ps_sum = ps_small.tile([1, QB], F32, tag="ps_small")
                n_red = nkp // 8
                red_i = 0
                level1 = []
                level2 = []
                level3 = []
                for i, kp in enumerate(range(kp0, kp0 + nkp)):
                    ex = pre[i] if pre is not None else scores_exp(qb, kp)
                    for half in range(2):
                        k = 2 * kp + half
                        nc.tensor.matmul(
                            ps_o[:],
                            lhsT=v_sb[:, k * D : (k + 1) * D],
                            rhs=ex[:, half * QB : (half + 1) * QB],
                            start=(kp == kp0 and half == 0),
                            stop=(kp == kp0 + nkp - 1 and half == 1),
                        )
                    comb = comb_pool.tile([128, QB], BF16, tag="comb")
                    nc.vector.tensor_add(comb[:], ex[:, 0:QB], ex[:, QB : 2 * QB])
                    level1.append(comb)
                    if len(level1) == 2:
                        comb2 = comb_pool.tile([128, QB], BF16, tag="comb")
                        nc.vector.tensor_add(comb2[:], level1[0][:], level1[1][:])
                        level1 = []
                        level2.append(comb2)
                        if len(level2) == 2:
                            comb3 = comb_pool.tile([128, QB], BF16, tag="comb")
                            nc.vector.tensor_add(comb3[:], level2[0][:], level2[1][:])
                            level2 = []
                            level3.append(comb3)
                            if len(level3) == 2:
                                comb4 = comb_pool.tile([128, QB], BF16, tag="comb")
                                nc.vector.tensor_add(
                                    comb4[:], level3[0][:], level3[1][:]
                                )
                                level3 = []
                                nc.tensor.matmul(
                                    ps_sum[:],
                                    lhsT=ones_col[:],
                                    rhs=comb4[:],
                                    start=(red_i == 0),
                                    stop=(red_i == n_red - 1),
                                )
                                red_i += 1
                o_sl = o_stage[:, qb * QB : (qb + 1) * QB]
                s_sl = sums_sb[:, qb * QB : (qb + 1) * QB]
                if first:
                    nc.vector.tensor_copy(o_sl, ps_o[:])
                    nc.vector.tensor_copy(s_sl, ps_sum[:])
                else:
                    nc.vector.tensor_add(o_sl, o_sl, ps_o[:])
                    nc.vector.tensor_add(s_sl, s_sl, ps_sum[:])
                if last:
                    nc.sync.dma_start(out_d[:, qb * QB : (qb + 1) * QB], o_sl)

            # ---- emission schedule ----
            # k own, v own (feeds the exchange), exchange, q, then attention
            for sq in range(NQ):
                project(1, kv_own, sq * SQ, sq, 1)
            for sq in range(NQ):
                project(2, kv_own, HALF + sq * SQ, sq, 2)

            # pairwise K/V exchange (overlapped with q proj + pass A)
            cc_in = dram_pool.tile([D, S], BF16, tag="cc_in")
            cc_out = dram_pool.tile([2, D, S], BF16, tag="cc_out")
            nc.sync.dma_start(cc_in[:], kv_own[:])
            nc.gpsimd.collective_compute(
                "AllGather",
                mybir.AluOpType.bypass,
                replica_groups=GROUPS,
                ins=[cc_in.opt()],
                outs=[cc_out.opt()],
            )
            nc.sync.dma_start(kv_peer[:], cc_out[peer_val])

            for sq in range(NQ):
                project(0, qT, sq * SQ, sq, 0)
            for k in range(16):  # own half v chunks
                v_transpose(k)

            # pass A: own chunks
            for qb in range(NQB):
                pv_sums(qb, 0, 8, first=True, last=False)
            for k in range(16, 32):  # peer v chunks
                v_transpose(k)
            # pass B: peer chunks
            for qb in range(NQB):
                pv_sums(qb, 8, 8, first=False, last=True)
            nc.sync.dma_start(sums_d[:], sums_sb[:])

    nc.compile()
    return nc


def _prep_inputs(x, W, b):
    """Host-side sharding prep: cast bf16, transpose to xT, slice halves."""
    b_f = np.asarray(b, dtype=np.float32)
    bias_cols = np.ascontiguousarray(b_f.reshape(3, D).T)  # [128, 3]
    w_bf = np.ascontiguousarray(
        np.asarray(W)
        .astype(ml_dtypes.bfloat16)
        .reshape(E // 128, 128, 3 * D)
        .transpose(1, 0, 2)
        .reshape(128, (E // 128) * 3 * D)
    )
    in_maps = []
    for bb in range(B):
        xt_full = np.ascontiguousarray(
            np.asarray(x[bb]).astype(ml_dtypes.bfloat16).T
        )  # [E, S]
        for h in range(2):
            xc = np.ascontiguousarray(xt_full[:, h * HALF : (h + 1) * HALF])
            peer = np.array([[1 - h]], dtype=np.uint32)
            in_maps.append(
                {"xt": xc, "w": w_bf, "bias_cols": bias_cols, "peer": peer}
            )
    return in_maps


def _run(in_maps, trace=False, trace_kwargs=None):
    if "nc" not in _CACHE:
        _CACHE["nc"] = _build()
    return run_bass_kernel_spmd(
        _CACHE["nc"],
        in_maps,
        list(range(N_CORES)),
        trace=trace,
        **(trace_kwargs or {}),
    )


def kernel(x, W, b):
    in_maps = _prep_inputs(x, W, b)
    res = None
    for attempt in range(3):
        try:
            res = _run(in_maps)
            break
        except Exception:
            if attempt == 2:
                raise
    out = np.empty((B, S, D), dtype=np.float32)
    for c in range(N_CORES):
        bb, h = c // 2, c % 2
        o_t = res.results[c]["out_t"]  # [D, HALF]
        sums = res.results[c]["sums"]  # [1, HALF]
        out[bb, h * HALF : (h + 1) * HALF, :] = (o_t / sums).T
    return out
